# revision 6
# baseline (speedup 1.0000x reference)
"""Bidirectional attention kernel for Trainium2 (8 NeuronCores, data-parallel over batch).

Math (per batch b):
  sim[l, m] = v1[l] . v2[m]                  (fp16 matmuls, [l,m] layout)
  Eb[l, m]  = exp(sim - 88)                  (no mask in exp; fixed softmax shift)
  Ea[m, l]  = Eb^T                           (DMA xbar transpose, bf16 bytes)
  out1[l] = (sum_m Ea[m,l] * v2e[m]) / S1[l] * keep1[l]
  out2[m] = (sum_l Eb[l,m] * v1e[l]) / S2[m] * keep2[m]

Masking is folded into the host-precomputed moving operands:
  v1e = [v1 * keep1, keep1, 0]  (keep1 = 1 - v1_mask) so masked-l rows drop
  out of both the numerator and the ones-column denominator of out2;
  likewise v2e for out1. The reference's masked entries carry weight
  exp(MASK_FILL)=1 against softmax row maxima of e^40+, a relative
  contribution < e^-30, far below the bf16 noise floor, so dropping them
  exactly matches the reference within tolerance.

The fixed shift (exp(x-88) instead of exp(x-max)) is safe: |sim| <~ 91,
and underflow terms are negligible relative to row sums.

All per-batch inputs ship as ONE merged DMA (fp16 container + bitcast
views) and both outputs leave as ONE merged bf16 DMA; the transpose runs
as 4 xbar instructions (2 chunks each, 3 on SP / 1 on ACT). Few DMA
instructions per batch keeps the tile framework's recycled DMA-semaphore
pool from serializing the pipeline.
"""
import sys
import types

import ml_dtypes
import numpy as np
from contextlib import ExitStack


def _install_axon_hooks_shim():
    """Provide antenv.axon_hooks if the image's antenv package lacks it
    (the axon boot shim needs it to register the NTFF profiling hook)."""
    if "antenv.axon_hooks" in sys.modules:
        return
    try:
        import antenv.axon_hooks  # noqa: F401
        return
    except ImportError:
        pass
    mod = types.ModuleType("antenv.axon_hooks")
    mod._hook = None

    def set_axon_ntff_profile_hook(hook):
        mod._hook = hook

    def get_axon_ntff_profile_hook():
        return mod._hook

    mod.set_axon_ntff_profile_hook = set_axon_ntff_profile_hook
    mod.get_axon_ntff_profile_hook = get_axon_ntff_profile_hook
    sys.modules["antenv.axon_hooks"] = mod
    try:
        import antenv

        antenv.axon_hooks = mod
    except ImportError:
        pass


_install_axon_hooks_shim()

import concourse.bacc as bacc
import concourse.mybir as mybir
import concourse.tile as tile
from concourse.bass_utils import run_bass_kernel_spmd

F32 = mybir.dt.float32
BF16 = mybir.dt.bfloat16
FP16 = mybir.dt.float16
AF = mybir.ActivationFunctionType

B, L, D = 64, 1024, 256
NCORES = 8
BPC = B // NCORES          # batches per core
NCH = L // 128             # 8 chunks of 128 along l or m
C_SHIFT = np.float32(88.0)
EW = D + 2  # 258: col 256 = keep-scaled ones (denominator), col 257 = pad

# merged input container offsets (fp16/2-byte units per partition)
OFF_V1T = 0
OFF_V2T = 2 * L                       # 2048
OFF_V1E = 4 * L                       # 4096
OFF_V2E = OFF_V1E + NCH * EW          # 6160
OFF_MV = OFF_V2E + NCH * EW           # 8224  (byte offset 16448, 4-aligned)
NIN = OFF_MV + 18 * 2                 # 8260 fp16 elems (mv: 18 f32)


def build_nc():
    nc = bacc.Bacc("TRN2", target_bir_lowering=False, debug=False)

    inb = nc.dram_tensor("inb", [BPC, 128, NIN], FP16, kind="ExternalInput").ap()
    outb = nc.dram_tensor(
        "outb", [BPC, 128, 2 * NCH * D], BF16, kind="ExternalOutput").ap()

    with tile.TileContext(nc) as tc, ExitStack() as ctx:
        warm_pool = ctx.enter_context(tc.tile_pool(name="warm", bufs=1))
        in_pool = ctx.enter_context(tc.tile_pool(name="inp", bufs=2))
        e_pool = ctx.enter_context(tc.tile_pool(name="epool", bufs=2))
        out_pool = ctx.enter_context(tc.tile_pool(name="outp", bufs=2))
        small_pool = ctx.enter_context(tc.tile_pool(name="small", bufs=4))
        psb_pool = ctx.enter_context(tc.tile_pool(name="psb", bufs=2, space="PSUM"))
        pso_pool = ctx.enter_context(tc.tile_pool(name="pso", bufs=4, space="PSUM"))

        # PE warmup: dummy matmuls while the first batch's input DMA
        # streams in, so the HAM clock-gate is at 2.4 GHz for real work.
        wz = warm_pool.tile([128, 512], BF16)
        nc.vector.memset(wz[:], 0.0)
        warmps = pso_pool.tile([128, EW], F32, tag="psO")
        for _ in range(28):
            nc.tensor.matmul(warmps[:], wz[:, 0:128], wz[:, 0:EW], start=True, stop=True)

        def emit_in_dma(b):
            in_sb = in_pool.tile([128, NIN], FP16, tag="inb", name="in_sb")
            nc.sync.dma_start(in_sb[:], inb[b])
            return in_sb

        cur = emit_in_dma(0)

        for b in range(BPC):
            in_sb = cur
            v1e_v = in_sb[:, OFF_V1E:OFF_V1E + NCH * EW].bitcast(BF16)
            v2e_v = in_sb[:, OFF_V2E:OFF_V2E + NCH * EW].bitcast(BF16)
            mv_v = in_sb[:, OFF_MV:NIN].bitcast(F32)

            Eb_sb = e_pool.tile([128, NCH * L], BF16, tag="Eb", name="Eb_sb")
            Ea_sb = e_pool.tile([128, NCH * L], BF16, tag="Ea", name="Ea_sb")
            out_sb = out_pool.tile([128, 2 * NCH * D], BF16, tag="ob", name="out_sb")

            # ---- sim + exp per l-chunk; xbar transposes in 2-chunk pairs
            for lc in range(NCH):
                psB = psb_pool.tile([128, L], F32, tag="psB", name="psB")
                for k in range(2):
                    for mh in range(2):
                        nc.tensor.matmul(
                            psB[:, mh * 512:(mh + 1) * 512],
                            in_sb[:, OFF_V1T + k * L + lc * 128:
                                  OFF_V1T + k * L + (lc + 1) * 128],
                            in_sb[:, OFF_V2T + k * L + mh * 512:
                                  OFF_V2T + k * L + mh * 512 + 512],
                            start=(k == 0),
                            stop=(k == 1),
                        )
                nc.scalar.activation(
                    Eb_sb[:, lc * L:(lc + 1) * L], psB[:], AF.Exp,
                    bias=mv_v[:, 16:17],
                )
                if lc in (1, 3, 5):
                    lo = lc - 1
                    nc.sync.dma_start_transpose(
                        Ea_sb[:, lo * L:(lo + 2) * L].rearrange(
                            "p (c j) -> p c j", c=2 * NCH),
                        Eb_sb[:, lo * L:(lo + 2) * L],
                    )
            nc.scalar.dma_start_transpose(
                Ea_sb[:, 6 * L:8 * L].rearrange("p (c j) -> p c j", c=2 * NCH),
                Eb_sb[:, 6 * L:8 * L],
            )

            # ---- attends, interleaved by data readiness:
            # aA(c) needs xbar pair c//2; aB(c) needs all exps.
            def attend(side, c):
                if side == "A":
                    stat, mov, kcol, oof = Ea_sb, v2e_v, 0, 0
                    # Ea layout: [m_p, c*L + k*128 + j]
                    slices = [
                        (c * L + k * 128, c * L + (k + 1) * 128) for k in range(NCH)]
                else:
                    stat, mov, kcol, oof = Eb_sb, v1e_v, 8, NCH * D
                    slices = [
                        (k * L + c * 128, k * L + (c + 1) * 128) for k in range(NCH)]
                psO = pso_pool.tile([128, EW], F32, tag="psO", name="psO")
                for k in range(NCH):
                    s0, s1 = slices[k]
                    nc.tensor.matmul(
                        psO[:], stat[:, s0:s1], mov[:, k * EW:(k + 1) * EW],
                        start=(k == 0), stop=(k == NCH - 1),
                    )
                rec = small_pool.tile([128, 1], F32, tag="rec", name="rec")
                cmb = small_pool.tile([128, 1], F32, tag="cmb", name="cmb")
                nc.vector.reciprocal(rec[:], psO[:, D:D + 1])
                nc.vector.tensor_mul(cmb[:], rec[:], mv_v[:, kcol + c: kcol + c + 1])
                nc.vector.tensor_scalar_mul(
                    out_sb[:, oof + c * D: oof + (c + 1) * D], psO[:, 0:D], cmb[:]
                )

            order = [("A", 0), ("A", 1), ("B", 0), ("A", 2), ("B", 1),
                     ("A", 3), ("B", 2), ("A", 4), ("B", 3), ("A", 5),
                     ("B", 4), ("A", 6), ("B", 5), ("A", 7), ("B", 6),
                     ("B", 7)]
            for side, c in order:
                attend(side, c)

            if b + 1 < BPC:
                cur = emit_in_dma(b + 1)
            nc.scalar.dma_start(outb[b], out_sb[:])

    nc.compile()
    return nc


def _prep_core_inputs(v1c, m1c, v2c, m2c):
    """v1c [BPC, L, D] f32, m1c [BPC, L] bool -> per-core input map."""
    f32 = np.float32
    bf = ml_dtypes.bfloat16
    nb = v1c.shape[0]
    # [nb, 128, 2, L] fp16: partition = d%128, then (k, l)
    v1t = v1c.transpose(0, 2, 1).reshape(nb, 2, 128, L).transpose(0, 2, 1, 3)
    v2t = v2c.transpose(0, 2, 1).reshape(nb, 2, 128, L).transpose(0, 2, 1, 3)
    v1t = np.ascontiguousarray(v1t).astype(np.float16).reshape(nb, 128, 2 * L)
    v2t = np.ascontiguousarray(v2t).astype(np.float16).reshape(nb, 128, 2 * L)

    keep1 = (1.0 - m1c.astype(f32))[:, :, None]          # [nb, L, 1]
    keep2 = (1.0 - m2c.astype(f32))[:, :, None]
    zeros = np.zeros((nb, L, 1), f32)
    v1e = np.concatenate([v1c * keep1, keep1, zeros], axis=2)
    v1e = v1e.reshape(nb, NCH, 128, EW).transpose(0, 2, 1, 3)
    v1e = np.ascontiguousarray(v1e).astype(bf).reshape(nb, 128, NCH * EW)
    v2e = np.concatenate([v2c * keep2, keep2, zeros], axis=2)
    v2e = v2e.reshape(nb, NCH, 128, EW).transpose(0, 2, 1, 3)
    v2e = np.ascontiguousarray(v2e).astype(bf).reshape(nb, 128, NCH * EW)

    mv = np.zeros((nb, 128, 18), f32)
    mv[:, :, 0:NCH] = keep1[:, :, 0].reshape(nb, NCH, 128).transpose(0, 2, 1)
    mv[:, :, 8:8 + NCH] = keep2[:, :, 0].reshape(nb, NCH, 128).transpose(0, 2, 1)
    mv[:, :, 16] = -C_SHIFT                              # exp bias column

    inb = np.empty((nb, 128, NIN), np.float16)
    inb[:, :, OFF_V1T:OFF_V2T] = v1t
    inb[:, :, OFF_V2T:OFF_V1E] = v2t
    inb[:, :, OFF_V1E:OFF_V2E] = v1e.view(np.float16)
    inb[:, :, OFF_V2E:OFF_MV] = v2e.view(np.float16)
    inb[:, :, OFF_MV:NIN] = mv.view(np.float16).reshape(nb, 128, 36)
    return {"inb": inb}


def run_on_hw(v1, v1_mask, v2, v2_mask, trace=False, nc=None):
    if nc is None:
        nc = build_nc()
    in_maps = []
    for i in range(NCORES):
        sl = slice(i * BPC, (i + 1) * BPC)
        in_maps.append(_prep_core_inputs(v1[sl], v1_mask[sl], v2[sl], v2_mask[sl]))
    res = run_bass_kernel_spmd(nc, in_maps, core_ids=list(range(NCORES)), trace=trace)
    a1 = np.empty((B, L, D), np.float32)
    a2 = np.empty((B, L, D), np.float32)
    for i, r in enumerate(res.results):
        sl = slice(i * BPC, (i + 1) * BPC)
        ob = r["outb"].reshape(BPC, 128, 2, NCH, D).astype(np.float32)
        a1[sl] = ob[:, :, 0].transpose(0, 2, 1, 3).reshape(BPC, L, D)
        a2[sl] = ob[:, :, 1].transpose(0, 2, 1, 3).reshape(BPC, L, D)
    return (a1, a2), res


def kernel(v1, v1_mask, v2, v2_mask):
    v1 = np.asarray(v1, np.float32)
    v2 = np.asarray(v2, np.float32)
    v1_mask = np.asarray(v1_mask)
    v2_mask = np.asarray(v2_mask)
    (a1, a2), _ = run_on_hw(v1, v1_mask, v2, v2_mask, trace=False)
    return a1, a2


# revision 9
# speedup vs baseline: 1.3638x; 1.3638x over previous
"""Bidirectional attention kernel for Trainium2 (8 NeuronCores, data-parallel over batch).

Math (per batch b):
  sim[l, m] = v1[l] . v2[m]                  (fp16 matmuls, [l,m] layout)
  Eb[l, m]  = exp(sim - 88)                  (no mask in exp; fixed softmax shift)
  Ea[m, l]  = Eb^T                           (DMA xbar transpose, bf16 bytes)
  out1[l] = (sum_m Ea[m,l] * v2e[m]) / S1[l] * keep1[l]
  out2[m] = (sum_l Eb[l,m] * v1e[l]) / S2[m] * keep2[m]

Masking is folded into the host-precomputed moving operands:
  v1e = [v1 * keep1, keep1, 0]  (keep1 = 1 - v1_mask) so masked-l rows drop
  out of both the numerator and the ones-column denominator of out2;
  likewise v2e for out1. The reference's masked entries carry weight
  exp(MASK_FILL)=1 against softmax row maxima of e^40+, a relative
  contribution < e^-30, far below the bf16 noise floor, so dropping them
  exactly matches the reference within tolerance.

The fixed shift (exp(x-88) instead of exp(x-max)) is safe: |sim| <~ 91,
and underflow terms are negligible relative to row sums.

All per-batch inputs ship as ONE merged DMA (fp16 container + bitcast
views) and both outputs leave as ONE merged bf16 DMA; the transpose runs
as 4 xbar instructions (2 chunks each, 3 on SP / 1 on ACT). Few DMA
instructions per batch keeps the tile framework's recycled DMA-semaphore
pool from serializing the pipeline.
"""
import sys
import types

import ml_dtypes
import numpy as np
from contextlib import ExitStack


def _install_axon_hooks_shim():
    """Provide antenv.axon_hooks if the image's antenv package lacks it
    (the axon boot shim needs it to register the NTFF profiling hook)."""
    if "antenv.axon_hooks" in sys.modules:
        return
    try:
        import antenv.axon_hooks  # noqa: F401
        return
    except ImportError:
        pass
    mod = types.ModuleType("antenv.axon_hooks")
    mod._hook = None

    def set_axon_ntff_profile_hook(hook):
        mod._hook = hook

    def get_axon_ntff_profile_hook():
        return mod._hook

    mod.set_axon_ntff_profile_hook = set_axon_ntff_profile_hook
    mod.get_axon_ntff_profile_hook = get_axon_ntff_profile_hook
    sys.modules["antenv.axon_hooks"] = mod
    try:
        import antenv

        antenv.axon_hooks = mod
    except ImportError:
        pass


_install_axon_hooks_shim()

import concourse.bacc as bacc
import concourse.mybir as mybir
import concourse.tile as tile
from concourse.bass_utils import run_bass_kernel_spmd

F32 = mybir.dt.float32
BF16 = mybir.dt.bfloat16
FP16 = mybir.dt.float16
AF = mybir.ActivationFunctionType

B, L, D = 64, 1024, 256
NCORES = 8
BPC = B // NCORES          # batches per core
NCH = L // 128             # 8 chunks of 128 along l or m
C_SHIFT = np.float32(88.0)
EW = D + 2  # 258: col 256 = keep-scaled ones (denominator), col 257 = pad

# merged input container offsets (fp16/2-byte units per partition)
OFF_V1T = 0
OFF_V2T = 2 * L                       # 2048
OFF_V1E = 4 * L                       # 4096
OFF_V2E = OFF_V1E + NCH * EW          # 6160
OFF_MV = OFF_V2E + NCH * EW           # 8224  (byte offset 16448, 4-aligned)
NIN = OFF_MV + 18 * 2                 # 8260 fp16 elems (mv: 18 f32)


def build_nc():
    nc = bacc.Bacc("TRN2", target_bir_lowering=False, debug=False)

    inb = nc.dram_tensor("inb", [BPC, 128, NIN], FP16, kind="ExternalInput").ap()
    outb = nc.dram_tensor(
        "outb", [BPC, 128, 2 * NCH * D], BF16, kind="ExternalOutput").ap()

    with tile.TileContext(nc) as tc, ExitStack() as ctx:
        warm_pool = ctx.enter_context(tc.tile_pool(name="warm", bufs=1))
        in_pool = ctx.enter_context(tc.tile_pool(name="inp", bufs=3))
        e_pool = ctx.enter_context(tc.tile_pool(name="epool", bufs=2))
        out_pool = ctx.enter_context(tc.tile_pool(name="outp", bufs=2))
        small_pool = ctx.enter_context(tc.tile_pool(name="small", bufs=4))
        psb_pool = ctx.enter_context(tc.tile_pool(name="psb", bufs=2, space="PSUM"))
        pso_pool = ctx.enter_context(tc.tile_pool(name="pso", bufs=4, space="PSUM"))

        # PE warmup: dummy matmuls while the first batch's input DMA
        # streams in, so the HAM clock-gate is at 2.4 GHz for real work.
        wz = warm_pool.tile([128, 512], BF16)
        nc.vector.memset(wz[:], 0.0)
        warmps = pso_pool.tile([128, EW], F32, tag="psO")
        for _ in range(28):
            nc.tensor.matmul(warmps[:], wz[:, 0:128], wz[:, 0:EW], start=True, stop=True)

        def emit_in_dma(b):
            in_sb = in_pool.tile([128, NIN], FP16, tag="inb", name="in_sb")
            nc.sync.dma_start(in_sb[:], inb[b])
            return in_sb

        cur = emit_in_dma(0)
        prev = None  # (in_sb, Eb_sb, Ea_sb) of batch b-1

        # attends for batch bp using its saved tiles (one-batch deferral:
        # xbars(b) serialize after aA(b-1) via sem recycling, and hide
        # under sims(b+1)+aB(b) before aA(b) needs them)
        def attends(bp, in_p, Eb_p, Ea_p):
            v1e_v = in_p[:, OFF_V1E:OFF_V1E + NCH * EW].bitcast(BF16)
            v2e_v = in_p[:, OFF_V2E:OFF_V2E + NCH * EW].bitcast(BF16)
            mv_v = in_p[:, OFF_MV:NIN].bitcast(F32)
            out_sb = out_pool.tile([128, 2 * NCH * D], BF16, tag="ob", name="out_sb")

            def attend(side, c):
                if side == "A":
                    stat, mov, kcol, oof = Ea_p, v2e_v, 0, 0
                    # Ea layout: [m_p, c*L + k*128 + j]
                    slices = [
                        (c * L + k * 128, c * L + (k + 1) * 128) for k in range(NCH)]
                else:
                    stat, mov, kcol, oof = Eb_p, v1e_v, 8, NCH * D
                    slices = [
                        (k * L + c * 128, k * L + (c + 1) * 128) for k in range(NCH)]
                psO = pso_pool.tile([128, EW], F32, tag="psO", name="psO")
                for k in range(NCH):
                    s0, s1 = slices[k]
                    nc.tensor.matmul(
                        psO[:], stat[:, s0:s1], mov[:, k * EW:(k + 1) * EW],
                        start=(k == 0), stop=(k == NCH - 1),
                    )
                rec = small_pool.tile([128, 1], F32, tag="rec", name="rec")
                cmb = small_pool.tile([128, 1], F32, tag="cmb", name="cmb")
                nc.vector.reciprocal(rec[:], psO[:, D:D + 1])
                nc.vector.tensor_mul(cmb[:], rec[:], mv_v[:, kcol + c: kcol + c + 1])
                nc.vector.tensor_scalar_mul(
                    out_sb[:, oof + c * D: oof + (c + 1) * D], psO[:, 0:D], cmb[:]
                )

            for c in range(NCH):
                attend("B", c)
                attend("A", c)
            nc.scalar.dma_start(outb[bp], out_sb[:])

        for b in range(BPC + 1):
            if b < BPC:
                in_sb = cur
                mv_v = in_sb[:, OFF_MV:NIN].bitcast(F32)
                Eb_sb = e_pool.tile([128, NCH * L], BF16, tag="Eb", name="Eb_sb")
                Ea_sb = e_pool.tile([128, NCH * L], BF16, tag="Ea", name="Ea_sb")

                # ---- sim + exp per l-chunk; xbar transposes in 2-chunk pairs
                for lc in range(NCH):
                    psB = psb_pool.tile([128, L], F32, tag="psB", name="psB")
                    for k in range(2):
                        for mh in range(2):
                            nc.tensor.matmul(
                                psB[:, mh * 512:(mh + 1) * 512],
                                in_sb[:, OFF_V1T + k * L + lc * 128:
                                      OFF_V1T + k * L + (lc + 1) * 128],
                                in_sb[:, OFF_V2T + k * L + mh * 512:
                                      OFF_V2T + k * L + mh * 512 + 512],
                                start=(k == 0),
                                stop=(k == 1),
                            )
                    nc.scalar.activation(
                        Eb_sb[:, lc * L:(lc + 1) * L], psB[:], AF.Exp,
                        bias=mv_v[:, 16:17],
                    )
                    if lc in (1, 3, 5, 7):
                        # all xbars on ONE queue (SP): concurrent xbar
                        # instructions from different queues corrupt each
                        # other on the shared xbar block.
                        lo = lc - 1
                        nc.sync.dma_start_transpose(
                            Ea_sb[:, lo * L:(lo + 2) * L].rearrange(
                                "p (c j) -> p c j", c=2 * NCH),
                            Eb_sb[:, lo * L:(lo + 2) * L],
                        )
                if b + 1 < BPC:
                    cur = emit_in_dma(b + 1)

            if prev is not None:
                attends(b - 1, *prev)
            if b < BPC:
                prev = (in_sb, Eb_sb, Ea_sb)

    nc.compile()
    return nc


def _prep_core_inputs(v1c, m1c, v2c, m2c):
    """v1c [BPC, L, D] f32, m1c [BPC, L] bool -> per-core input map."""
    f32 = np.float32
    bf = ml_dtypes.bfloat16
    nb = v1c.shape[0]
    # [nb, 128, 2, L] fp16: partition = d%128, then (k, l)
    v1t = v1c.transpose(0, 2, 1).reshape(nb, 2, 128, L).transpose(0, 2, 1, 3)
    v2t = v2c.transpose(0, 2, 1).reshape(nb, 2, 128, L).transpose(0, 2, 1, 3)
    v1t = np.ascontiguousarray(v1t).astype(np.float16).reshape(nb, 128, 2 * L)
    v2t = np.ascontiguousarray(v2t).astype(np.float16).reshape(nb, 128, 2 * L)

    keep1 = (1.0 - m1c.astype(f32))[:, :, None]          # [nb, L, 1]
    keep2 = (1.0 - m2c.astype(f32))[:, :, None]
    zeros = np.zeros((nb, L, 1), f32)
    v1e = np.concatenate([v1c * keep1, keep1, zeros], axis=2)
    v1e = v1e.reshape(nb, NCH, 128, EW).transpose(0, 2, 1, 3)
    v1e = np.ascontiguousarray(v1e).astype(bf).reshape(nb, 128, NCH * EW)
    v2e = np.concatenate([v2c * keep2, keep2, zeros], axis=2)
    v2e = v2e.reshape(nb, NCH, 128, EW).transpose(0, 2, 1, 3)
    v2e = np.ascontiguousarray(v2e).astype(bf).reshape(nb, 128, NCH * EW)

    mv = np.zeros((nb, 128, 18), f32)
    mv[:, :, 0:NCH] = keep1[:, :, 0].reshape(nb, NCH, 128).transpose(0, 2, 1)
    mv[:, :, 8:8 + NCH] = keep2[:, :, 0].reshape(nb, NCH, 128).transpose(0, 2, 1)
    mv[:, :, 16] = -C_SHIFT                              # exp bias column

    inb = np.empty((nb, 128, NIN), np.float16)
    inb[:, :, OFF_V1T:OFF_V2T] = v1t
    inb[:, :, OFF_V2T:OFF_V1E] = v2t
    inb[:, :, OFF_V1E:OFF_V2E] = v1e.view(np.float16)
    inb[:, :, OFF_V2E:OFF_MV] = v2e.view(np.float16)
    inb[:, :, OFF_MV:NIN] = mv.view(np.float16).reshape(nb, 128, 36)
    return {"inb": inb}


def run_on_hw(v1, v1_mask, v2, v2_mask, trace=False, nc=None):
    if nc is None:
        nc = build_nc()
    in_maps = []
    for i in range(NCORES):
        sl = slice(i * BPC, (i + 1) * BPC)
        in_maps.append(_prep_core_inputs(v1[sl], v1_mask[sl], v2[sl], v2_mask[sl]))
    res = run_bass_kernel_spmd(nc, in_maps, core_ids=list(range(NCORES)), trace=trace)
    a1 = np.empty((B, L, D), np.float32)
    a2 = np.empty((B, L, D), np.float32)
    for i, r in enumerate(res.results):
        sl = slice(i * BPC, (i + 1) * BPC)
        ob = r["outb"].reshape(BPC, 128, 2, NCH, D).astype(np.float32)
        a1[sl] = ob[:, :, 0].transpose(0, 2, 1, 3).reshape(BPC, L, D)
        a2[sl] = ob[:, :, 1].transpose(0, 2, 1, 3).reshape(BPC, L, D)
    return (a1, a2), res


def kernel(v1, v1_mask, v2, v2_mask):
    v1 = np.asarray(v1, np.float32)
    v2 = np.asarray(v2, np.float32)
    v1_mask = np.asarray(v1_mask)
    v2_mask = np.asarray(v2_mask)
    (a1, a2), _ = run_on_hw(v1, v1_mask, v2, v2_mask, trace=False)
    return a1, a2
